# revision 6
# baseline (speedup 1.0000x reference)
"""LSTM decoder kernel for Trainium2 (8 NeuronCores, pure data parallel).

Problem: 25-step autoregressive LSTM decode, BATCH=262144, POSE=16, H=32.
  reference: per step  gates = x@W_ih.T + h@W_hh.T + b;  i,f,g,o = split(gates)
             c = sig(f)*c + sig(i)*tanh(g); h = sig(o)*tanh(c); x = h@W_out.T + b_out

Design (v2) — engine-balance around the ScalarE(ACT) drain bound:
  * Projection folded into the recurrence (W_eff = W_ih@W_out + W_hh), so each
    step needs one K=32 matmul per gate type; the pose outputs are recovered on
    the HOST from the h-sequence (x_t = h_t @ W_out.T + b_out), which the
    kernel streams to DRAM in bf16.
  * Strip layout: state lives as [128 = 4 strips x 32 feats, cols]; batch row
    (g, x, j) = g*2048 + x*512 + j  <->  partition 32x+k, col 512g+j.
  * Gate matmuls: BLOCK-DIAGONAL stationary [128,128] (4 copies of
    W_eff_ty.T on the diagonal) -> one matmul per (gate type, group of 2048
    rows); gates land strip-aligned in PSUM [128, 2048] per type per block of
    4 groups.  K=128 fully used; 1 batch row/cycle.
  * ACT drains+activates each gate tensor in ONE FD=2048 instruction
    (sigmoid/tanh with per-partition bias).  This is the structural bottleneck
    (~1.9us/tensor); tanh(c) is therefore split: 87.5% of columns evaluated on
    the DVE as a clamped degree-7 odd polynomial (quadratic x linear factored,
    fused scalar_tensor_tensor ops), the rest on ACT.
  * sig(i)*tanh(g) product + the clamp run on GpSimd; the rest of the cell
    update (f*c, +, o*tanh(c)) on DVE, all bf16 (2x mode).
  * dtypes: matmuls/state bf16, PSUM f32; end-to-end rel l2 ~ 7e-3.
"""

import numpy as np
import ml_dtypes

bf16 = ml_dtypes.bfloat16

H = 32
PD = 16
SEQ = 25
BATCH = 262144
NCORES = 8

B_LOC = BATCH // NCORES          # 32768 rows per core
NB = 512                         # cols per group-chunk (one strip-group)
GROUPS = B_LOC // (4 * NB)       # 16 groups of 2048 rows
BLOCKS = 4                       # process 4 groups per wave => FD=2048
FD = (GROUPS // BLOCKS) * NB     # 2048
CALL = GROUPS * NB               # 8192 state cols per core

# tanh(c) ~ xc*(q + delta), q = (u + beta)*z, u = (t + alpha)*t,
# z = c7*t + gamma, t = xc^2, xc = clamp(c, +-XCLAMP).  deg-7 minimax-ish fit
# on [0, 2.5]; end-to-end rel l2 validated at 7.3e-3.
XCLAMP = 2.5
_C1, _C3, _C5, _C7 = 0.9700571610631279, -0.23796964748377925, \
    0.03875110981187037, -0.0024742524169626846
ALPHA = -8.0
GAMMA = _C5 - ALPHA * _C7
BETA = (_C3 - ALPHA * GAMMA) / _C7
DELTA = _C1 - BETA * GAMMA

SD = 1792                        # cols of each 2048-block whose tanh(c) is DVE


def _f32(x):
    return np.ascontiguousarray(np.asarray(x, dtype=np.float32))


def _blkdiag(Wty, in_dim):
    """[32(out k), in_dim(m)] gate-type slice -> [128,128] block-diag lhsT.

    lhsT[32x+m, 32x+k] = Wty[k, m] for m < in_dim (zeros elsewhere).
    """
    out = np.zeros((128, 128), np.float32)
    blk = np.zeros((32, 32), np.float32)
    blk[:in_dim, :] = Wty.T[:in_dim, :]
    for x in range(4):
        out[32 * x : 32 * x + 32, 32 * x : 32 * x + 32] = blk
    return out


def prep_weights(W_ih, W_hh, b_ih, b_hh, W_out, b_out):
    W_ih, W_hh, b_ih, b_hh, W_out, b_out = map(
        _f32, (W_ih, W_hh, b_ih, b_hh, W_out, b_out)
    )
    b1 = b_ih + b_hh
    W_eff = W_ih @ W_out + W_hh            # [4H, H]
    b_eff = b1 + W_ih @ b_out

    def pack4(W, in_dim):
        # [128, 512]: cols 128*ty..+128 = block-diag lhsT for gate type ty
        return np.ascontiguousarray(np.concatenate(
            [_blkdiag(W[32 * ty : 32 * ty + 32, :], in_dim) for ty in range(4)],
            axis=1).astype(bf16))

    weff = pack4(W_eff, H)
    whh = pack4(W_hh, H)
    wih = pack4(W_ih, PD)

    bias = np.zeros((128, 8), np.float32)
    for ty in range(4):
        bias[:, ty] = np.tile(b1[32 * ty : 32 * ty + 32], 4)
        bias[:, 4 + ty] = np.tile(b_eff[32 * ty : 32 * ty + 32], 4)
    return dict(weff=weff, whh=whh, wih=wih, bias=bias,
                W_out=W_out, b_out=b_out)


def prep_state(arr, feat):
    """[B_LOC, feat] batch-major -> strip layout [128, CALL] bf16."""
    a = np.zeros((B_LOC, H), np.float32)
    a[:, :feat] = arr[:, :feat]
    a = a.reshape(GROUPS, 4, NB, H)           # [g, x, j, k]
    a = a.transpose(1, 3, 0, 2)               # [x, k, g, j]
    return np.ascontiguousarray(a.reshape(128, CALL).astype(bf16))


def build_nc():
    import concourse.bass as bass
    import concourse.bacc as bacc
    import concourse.mybir as mybir
    import concourse.tile as tile

    F32 = mybir.dt.float32
    BF16 = mybir.dt.bfloat16
    AF = mybir.ActivationFunctionType
    OP = mybir.AluOpType

    nc = bacc.Bacc("TRN2", target_bir_lowering=False, debug=False)
    hT_d = nc.declare_dram_parameter("hT", [128, CALL], BF16, isOutput=False)
    cT_d = nc.declare_dram_parameter("cT", [128, CALL], BF16, isOutput=False)
    xT_d = nc.declare_dram_parameter("xT", [128, CALL], BF16, isOutput=False)
    weff_d = nc.declare_dram_parameter("weff", [128, 512], BF16, isOutput=False)
    whh_d = nc.declare_dram_parameter("whh", [128, 512], BF16, isOutput=False)
    wih_d = nc.declare_dram_parameter("wih", [128, 512], BF16, isOutput=False)
    bias_d = nc.declare_dram_parameter("bias", [128, 8], F32, isOutput=False)
    hout_d = nc.declare_dram_parameter(
        "hout", [128, SEQ * BLOCKS * FD], BF16, isOutput=True)

    # wave order: f first (t1 needs it), then i, g (t2), o last
    WAVES = [(1, AF.Sigmoid), (0, AF.Sigmoid), (2, AF.Tanh), (3, AF.Sigmoid)]

    with tile.TileContext(nc) as tc:
        with (
            tc.tile_pool(name="const", bufs=1) as const,
            tc.tile_pool(name="hpool", bufs=3) as hpool,
            tc.tile_pool(name="gpsum", bufs=2, space=bass.MemorySpace.PSUM) as gpsum,
            tc.tile_pool(name="gate", bufs=2) as gate,
            tc.tile_pool(name="tmpa", bufs=2) as tmpa,
            tc.tile_pool(name="tmpb", bufs=2) as tmpb,
        ):
            weff_t = const.tile([128, 512], BF16)
            whh_t = const.tile([128, 512], BF16)
            wih_t = const.tile([128, 512], BF16)
            bias_t = const.tile([128, 8], F32)
            c_all = const.tile([128, CALL], BF16)
            x0_t = const.tile([128, CALL], BF16)
            nc.sync.dma_start(weff_t[:], weff_d[:])
            nc.sync.dma_start(whh_t[:], whh_d[:])
            nc.sync.dma_start(wih_t[:], wih_d[:])
            nc.sync.dma_start(bias_t[:], bias_d[:])
            nc.sync.dma_start(c_all[:], cT_d[:])
            nc.sync.dma_start(x0_t[:], xT_d[:])

            h_cur = hpool.tile([128, CALL], BF16, name="h")
            nc.sync.dma_start(h_cur[:], hT_d[:])

            for t in range(SEQ):
                h_next = hpool.tile([128, CALL], BF16, name="h")
                for b in range(BLOCKS):
                    c0 = b * FD
                    S = {}
                    for ty, func in WAVES:
                        P = gpsum.tile([128, FD], F32, name="P")
                        ws = slice(128 * ty, 128 * ty + 128)
                        for g4 in range(4):
                            g = 4 * b + g4
                            gc = slice(NB * g, NB * (g + 1))
                            ps = P[:, NB * g4 : NB * (g4 + 1)]
                            if t == 0:
                                nc.tensor.matmul(
                                    ps, whh_t[:, ws], h_cur[:, gc],
                                    start=True, stop=False)
                                nc.tensor.matmul(
                                    ps, wih_t[:, ws], x0_t[:, gc],
                                    start=False, stop=True)
                            else:
                                nc.tensor.matmul(
                                    ps, weff_t[:, ws], h_cur[:, gc],
                                    start=True, stop=True)
                        bcol = ty if t == 0 else 4 + ty
                        s_t = gate.tile([128, FD], BF16, name=f"s{ty}")
                        nc.scalar.activation(
                            s_t[:], P[:], func,
                            bias=bias_t[:, bcol : bcol + 1])
                        S[ty] = s_t
                    s_i, s_f, s_g, s_o = S[0], S[1], S[2], S[3]

                    cs = slice(c0, c0 + FD)
                    t1 = tmpa.tile([128, FD], BF16, name="t1")
                    t2 = tmpa.tile([128, FD], BF16, name="t2")
                    nc.vector.tensor_mul(t1[:], s_f[:], c_all[:, cs])
                    nc.gpsimd.tensor_mul(t2[:], s_i[:], s_g[:])
                    nc.vector.tensor_add(c_all[:, cs], t1[:], t2[:])

                    # tanh(c): cols [0,SD) via DVE poly, [SD,FD) via ACT
                    tc_t = tmpa.tile([128, FD], BF16, name="tc")
                    xc = tmpb.tile([128, SD], BF16, name="xc")
                    tt = tmpb.tile([128, SD], BF16, name="tt")
                    u = tmpb.tile([128, SD], BF16, name="u")
                    z = tmpb.tile([128, SD], BF16, name="z")
                    qq = tmpb.tile([128, SD], BF16, name="qq")
                    nc.gpsimd.tensor_scalar(
                        xc[:], c_all[:, c0 : c0 + SD],
                        float(XCLAMP), float(-XCLAMP), OP.min, OP.max)
                    nc.vector.tensor_mul(tt[:], xc[:], xc[:])
                    nc.vector.scalar_tensor_tensor(
                        u[:], tt[:], float(ALPHA), tt[:], OP.add, OP.mult)
                    nc.vector.tensor_scalar(
                        z[:], tt[:], float(_C7), float(GAMMA), OP.mult, OP.add)
                    nc.vector.scalar_tensor_tensor(
                        qq[:], u[:], float(BETA), z[:], OP.add, OP.mult)
                    nc.vector.scalar_tensor_tensor(
                        tc_t[:, :SD], qq[:], float(DELTA), xc[:],
                        OP.add, OP.mult)
                    nc.scalar.activation(
                        tc_t[:, SD:], c_all[:, c0 + SD : c0 + FD], AF.Tanh)

                    nc.vector.tensor_mul(h_next[:, cs], s_o[:], tc_t[:])
                    nc.sync.dma_start(
                        hout_d[:, (t * BLOCKS + b) * FD : (t * BLOCKS + b + 1) * FD],
                        h_next[:, cs])
                h_cur = h_next
    nc.compile()
    return nc


_NC_CACHE = {}


def _get_nc(key="v2"):
    if key not in _NC_CACHE:
        _NC_CACHE[key] = build_nc()
    return _NC_CACHE[key]


def make_in_maps(inputs):
    first_input = _f32(inputs["first_input"])
    h0 = _f32(inputs["h0"])
    c0 = _f32(inputs["c0"])
    w = prep_weights(
        inputs["W_ih"], inputs["W_hh"], inputs["b_ih"], inputs["b_hh"],
        inputs["W_out"], inputs["b_out"],
    )
    shared = dict(weff=w["weff"], whh=w["whh"], wih=w["wih"], bias=w["bias"])
    in_maps = []
    for ci in range(NCORES):
        rows = slice(ci * B_LOC, (ci + 1) * B_LOC)
        in_maps.append(dict(
            shared,
            hT=prep_state(h0[rows], H),
            cT=prep_state(c0[rows], H),
            xT=prep_state(first_input[rows], PD),
        ))
    return in_maps, w


def postprocess(results, w):
    """Per-core hout [128, SEQ*BLOCKS*FD] bf16 -> full [BATCH, SEQ, PD] f32."""
    W_outT = w["W_out"].T.astype(np.float32)       # [H, PD]
    b_out = w["b_out"].astype(np.float32)
    outs = []
    for ci in range(NCORES):
        a = np.asarray(results[ci]["hout"])
        # [128, SEQ*BLOCKS*FD] -> [x, k, t, b, g4, j]
        a = a.reshape(4, 32, SEQ, BLOCKS, 4, NB)
        # -> [b, g4, x, j, t, k]
        a = np.ascontiguousarray(a.transpose(3, 4, 0, 5, 2, 1))
        h = a.reshape(B_LOC * SEQ, H).astype(np.float32)
        x = h @ W_outT + b_out
        outs.append(x.reshape(B_LOC, SEQ, PD))
    return np.concatenate(outs, axis=0)


def kernel(**inputs) -> np.ndarray:
    from concourse.bass_utils import run_bass_kernel_spmd

    in_maps, w = make_in_maps(inputs)
    nc = _get_nc()
    res = run_bass_kernel_spmd(nc, in_maps, core_ids=list(range(NCORES)))
    return postprocess(res.results, w)


if __name__ == "__main__":
    nc = build_nc()
    n = sum(len(b.instructions) for b in nc.m.functions[0].blocks)
    print("built; instructions:", n)


# revision 9
# speedup vs baseline: 1.7185x; 1.7185x over previous
"""LSTM decoder kernel for Trainium2 (8 NeuronCores, pure data parallel).

Problem: 25-step autoregressive LSTM decode, BATCH=262144, POSE=16, H=32.
  reference: per step  gates = x@W_ih.T + h@W_hh.T + b;  i,f,g,o = split(gates)
             c = sig(f)*c + sig(i)*tanh(g); h = sig(o)*tanh(c); x = h@W_out.T + b_out

Design (v2) — engine-balance around the ScalarE(ACT) drain bound:
  * Projection folded into the recurrence (W_eff = W_ih@W_out + W_hh), so each
    step needs one K=32 matmul per gate type; the pose outputs are recovered on
    the HOST from the h-sequence (x_t = h_t @ W_out.T + b_out), which the
    kernel streams to DRAM in bf16.
  * Strip layout: state lives as [128 = 4 strips x 32 feats, cols]; batch row
    (g, x, j) = g*2048 + x*512 + j  <->  partition 32x+k, col 512g+j.
  * Gate matmuls: BLOCK-DIAGONAL stationary [128,128] (4 copies of
    W_eff_ty.T on the diagonal) -> one matmul per (gate type, group of 2048
    rows); gates land strip-aligned in PSUM [128, 2048] per type per block of
    4 groups.  K=128 fully used; 1 batch row/cycle.
  * ACT drains+activates each gate tensor in ONE FD=2048 instruction
    (sigmoid/tanh with per-partition bias).  This is the structural bottleneck
    (~1.9us/tensor); tanh(c) is therefore split: 87.5% of columns evaluated on
    the DVE as a clamped degree-7 odd polynomial (quadratic x linear factored,
    fused scalar_tensor_tensor ops), the rest on ACT.
  * sig(i)*tanh(g) product + the clamp run on GpSimd; the rest of the cell
    update (f*c, +, o*tanh(c)) on DVE, all bf16 (2x mode).
  * dtypes: matmuls/state bf16, PSUM f32; end-to-end rel l2 ~ 7e-3.
"""

import numpy as np
import ml_dtypes

bf16 = ml_dtypes.bfloat16

H = 32
PD = 16
SEQ = 25
BATCH = 262144
NCORES = 8

B_LOC = BATCH // NCORES          # 32768 rows per core
NB = 512                         # cols per group-chunk (one strip-group)
GROUPS = B_LOC // (4 * NB)       # 16 groups of 2048 rows
BLOCKS = 4                       # process 4 groups per wave => FD=2048
FD = (GROUPS // BLOCKS) * NB     # 2048
CALL = GROUPS * NB               # 8192 state cols per core

# tanh(c) ~ xc*(q + delta), q = (u + beta)*z, u = (t + alpha)*t,
# z = c7*t + gamma, t = xc^2, xc = clamp(c, +-XCLAMP).  deg-7 minimax-ish fit
# on [0, 2.5]; end-to-end rel l2 validated at 7.3e-3.
XCLAMP = 2.5
_C1, _C3, _C5, _C7 = 0.9700571610631279, -0.23796964748377925, \
    0.03875110981187037, -0.0024742524169626846
ALPHA = -8.0
GAMMA = _C5 - ALPHA * _C7
BETA = (_C3 - ALPHA * GAMMA) / _C7
DELTA = _C1 - BETA * GAMMA

SD = 1024                        # cols of each 2048-block whose tanh(c) is DVE


def _f32(x):
    return np.ascontiguousarray(np.asarray(x, dtype=np.float32))


def _blkdiag(Wty, in_dim):
    """[32(out k), in_dim(m)] gate-type slice -> [128,128] block-diag lhsT.

    lhsT[32x+m, 32x+k] = Wty[k, m] for m < in_dim (zeros elsewhere).
    """
    out = np.zeros((128, 128), np.float32)
    blk = np.zeros((32, 32), np.float32)
    blk[:in_dim, :] = Wty.T[:in_dim, :]
    for x in range(4):
        out[32 * x : 32 * x + 32, 32 * x : 32 * x + 32] = blk
    return out


def prep_weights(W_ih, W_hh, b_ih, b_hh, W_out, b_out):
    W_ih, W_hh, b_ih, b_hh, W_out, b_out = map(
        _f32, (W_ih, W_hh, b_ih, b_hh, W_out, b_out)
    )
    b1 = b_ih + b_hh
    W_eff = W_ih @ W_out + W_hh            # [4H, H]
    b_eff = b1 + W_ih @ b_out

    def pack4(W, in_dim):
        # [128, 512]: cols 128*ty..+128 = block-diag lhsT for gate type ty
        return np.ascontiguousarray(np.concatenate(
            [_blkdiag(W[32 * ty : 32 * ty + 32, :], in_dim) for ty in range(4)],
            axis=1).astype(bf16))

    weff = pack4(W_eff, H)
    whh = pack4(W_hh, H)
    wih = pack4(W_ih, PD)

    bias = np.zeros((128, 8), np.float32)
    for ty in range(4):
        bias[:, ty] = np.tile(b1[32 * ty : 32 * ty + 32], 4)
        bias[:, 4 + ty] = np.tile(b_eff[32 * ty : 32 * ty + 32], 4)
    return dict(weff=weff, whh=whh, wih=wih, bias=bias,
                W_out=W_out, b_out=b_out)


def prep_state(arr, feat):
    """[B_LOC, feat] batch-major -> strip layout [128, CALL] bf16."""
    a = np.zeros((B_LOC, H), np.float32)
    a[:, :feat] = arr[:, :feat]
    a = a.reshape(GROUPS, 4, NB, H)           # [g, x, j, k]
    a = a.transpose(1, 3, 0, 2)               # [x, k, g, j]
    return np.ascontiguousarray(a.reshape(128, CALL).astype(bf16))


def build_nc():
    import concourse.bass as bass
    import concourse.bacc as bacc
    import concourse.mybir as mybir
    import concourse.tile as tile

    F32 = mybir.dt.float32
    BF16 = mybir.dt.bfloat16
    AF = mybir.ActivationFunctionType
    OP = mybir.AluOpType

    nc = bacc.Bacc("TRN2", target_bir_lowering=False, debug=False)
    hT_d = nc.declare_dram_parameter("hT", [128, CALL], BF16, isOutput=False)
    cT_d = nc.declare_dram_parameter("cT", [128, CALL], BF16, isOutput=False)
    xT_d = nc.declare_dram_parameter("xT", [128, CALL], BF16, isOutput=False)
    weff_d = nc.declare_dram_parameter("weff", [128, 512], BF16, isOutput=False)
    whh_d = nc.declare_dram_parameter("whh", [128, 512], BF16, isOutput=False)
    wih_d = nc.declare_dram_parameter("wih", [128, 512], BF16, isOutput=False)
    bias_d = nc.declare_dram_parameter("bias", [128, 8], F32, isOutput=False)
    hout_d = nc.declare_dram_parameter(
        "hout", [128, SEQ * BLOCKS * FD], BF16, isOutput=True)

    # wave order: f first (t1 needs it), then i, g (t2), o last
    WAVES = [(1, AF.Sigmoid), (0, AF.Sigmoid), (2, AF.Tanh), (3, AF.Sigmoid)]

    with tile.TileContext(nc) as tc:
        with (
            tc.tile_pool(name="const", bufs=1) as const,
            tc.tile_pool(name="hpool", bufs=3) as hpool,
            tc.tile_pool(name="gpsum", bufs=2, space=bass.MemorySpace.PSUM) as gpsum,
            tc.tile_pool(name="gate", bufs=2) as gate,
            tc.tile_pool(name="tmpa", bufs=2) as tmpa,
            tc.tile_pool(name="tmpb", bufs=2) as tmpb,
        ):
            weff_t = const.tile([128, 512], BF16)
            whh_t = const.tile([128, 512], BF16)
            wih_t = const.tile([128, 512], BF16)
            bias_t = const.tile([128, 8], F32)
            c_all = const.tile([128, CALL], BF16)
            x0_t = const.tile([128, CALL], BF16)
            nc.sync.dma_start(weff_t[:], weff_d[:])
            nc.sync.dma_start(whh_t[:], whh_d[:])
            nc.sync.dma_start(wih_t[:], wih_d[:])
            nc.sync.dma_start(bias_t[:], bias_d[:])
            nc.sync.dma_start(c_all[:], cT_d[:])
            nc.sync.dma_start(x0_t[:], xT_d[:])

            h_cur = hpool.tile([128, CALL], BF16, name="h")
            nc.sync.dma_start(h_cur[:], hT_d[:])

            for t in range(SEQ):
                h_next = hpool.tile([128, CALL], BF16, name="h")
                for b in range(BLOCKS):
                    c0 = b * FD
                    S = {}
                    for ty, func in WAVES:
                        P = gpsum.tile([128, FD], F32, name="P")
                        ws = slice(128 * ty, 128 * ty + 128)
                        for g4 in range(4):
                            g = 4 * b + g4
                            gc = slice(NB * g, NB * (g + 1))
                            ps = P[:, NB * g4 : NB * (g4 + 1)]
                            if t == 0:
                                nc.tensor.matmul(
                                    ps, whh_t[:, ws], h_cur[:, gc],
                                    start=True, stop=False)
                                nc.tensor.matmul(
                                    ps, wih_t[:, ws], x0_t[:, gc],
                                    start=False, stop=True)
                            else:
                                nc.tensor.matmul(
                                    ps, weff_t[:, ws], h_cur[:, gc],
                                    start=True, stop=True)
                        bcol = ty if t == 0 else 4 + ty
                        s_t = gate.tile([128, FD], BF16, name=f"s{ty}")
                        nc.scalar.activation(
                            s_t[:], P[:], func,
                            bias=bias_t[:, bcol : bcol + 1])
                        S[ty] = s_t
                    s_i, s_f, s_g, s_o = S[0], S[1], S[2], S[3]

                    cs = slice(c0, c0 + FD)
                    t1 = tmpa.tile([128, FD], BF16, name="t1")
                    t2 = tmpa.tile([128, FD], BF16, name="t2")
                    nc.vector.tensor_mul(t1[:], s_f[:], c_all[:, cs])
                    nc.vector.tensor_mul(t2[:], s_i[:], s_g[:])
                    nc.vector.tensor_add(c_all[:, cs], t1[:], t2[:])

                    # tanh(c): cols [0,SD) DVE Horner (TT + 1-scalar TS only),
                    # cols [SD,FD) exact on ACT
                    tc_t = tmpa.tile([128, FD], BF16, name="tc")
                    if SD > 0:
                        xc = tmpb.tile([128, SD], BF16, name="xc")
                        x2 = tmpb.tile([128, SD], BF16, name="x2")
                        tt = tmpb.tile([128, SD], BF16, name="tt")
                        p1 = tmpb.tile([128, SD], BF16, name="p1")
                        p2 = tmpb.tile([128, SD], BF16, name="p2")
                        nc.vector.tensor_scalar_min(
                            x2[:], c_all[:, c0 : c0 + SD], float(XCLAMP))
                        nc.vector.tensor_scalar_max(
                            xc[:], x2[:], float(-XCLAMP))
                        nc.vector.tensor_mul(tt[:], xc[:], xc[:])
                        nc.vector.tensor_scalar_mul(p1[:], tt[:], float(_C7))
                        nc.vector.tensor_scalar_add(p2[:], p1[:], float(_C5))
                        nc.vector.tensor_mul(p1[:], p2[:], tt[:])
                        nc.vector.tensor_scalar_add(p2[:], p1[:], float(_C3))
                        nc.vector.tensor_mul(p1[:], p2[:], tt[:])
                        nc.vector.tensor_scalar_add(p2[:], p1[:], float(_C1))
                        nc.vector.tensor_mul(tc_t[:, :SD], p2[:], xc[:])
                    if SD < FD:
                        nc.scalar.activation(
                            tc_t[:, SD:], c_all[:, c0 + SD : c0 + FD], AF.Tanh)

                    nc.vector.tensor_mul(h_next[:, cs], s_o[:], tc_t[:])
                    nc.sync.dma_start(
                        hout_d[:, (t * BLOCKS + b) * FD : (t * BLOCKS + b + 1) * FD],
                        h_next[:, cs])
                h_cur = h_next
    nc.compile()
    return nc


_NC_CACHE = {}


def _get_nc(key="v2"):
    if key not in _NC_CACHE:
        _NC_CACHE[key] = build_nc()
    return _NC_CACHE[key]


def make_in_maps(inputs):
    first_input = _f32(inputs["first_input"])
    h0 = _f32(inputs["h0"])
    c0 = _f32(inputs["c0"])
    w = prep_weights(
        inputs["W_ih"], inputs["W_hh"], inputs["b_ih"], inputs["b_hh"],
        inputs["W_out"], inputs["b_out"],
    )
    shared = dict(weff=w["weff"], whh=w["whh"], wih=w["wih"], bias=w["bias"])
    in_maps = []
    for ci in range(NCORES):
        rows = slice(ci * B_LOC, (ci + 1) * B_LOC)
        in_maps.append(dict(
            shared,
            hT=prep_state(h0[rows], H),
            cT=prep_state(c0[rows], H),
            xT=prep_state(first_input[rows], PD),
        ))
    return in_maps, w


def postprocess(results, w):
    """Per-core hout [128, SEQ*BLOCKS*FD] bf16 -> full [BATCH, SEQ, PD] f32."""
    W_outT = w["W_out"].T.astype(np.float32)       # [H, PD]
    b_out = w["b_out"].astype(np.float32)
    outs = []
    for ci in range(NCORES):
        a = np.asarray(results[ci]["hout"])
        # [128, SEQ*BLOCKS*FD] -> [x, k, t, b, g4, j]
        a = a.reshape(4, 32, SEQ, BLOCKS, 4, NB)
        # -> [b, g4, x, j, t, k]
        a = np.ascontiguousarray(a.transpose(3, 4, 0, 5, 2, 1))
        h = a.reshape(B_LOC * SEQ, H).astype(np.float32)
        x = h @ W_outT + b_out
        outs.append(x.reshape(B_LOC, SEQ, PD))
    return np.concatenate(outs, axis=0)


def kernel(**inputs) -> np.ndarray:
    from concourse.bass_utils import run_bass_kernel_spmd

    in_maps, w = make_in_maps(inputs)
    nc = _get_nc()
    res = run_bass_kernel_spmd(nc, in_maps, core_ids=list(range(NCORES)))
    return postprocess(res.results, w)


if __name__ == "__main__":
    nc = build_nc()
    n = sum(len(b.instructions) for b in nc.m.functions[0].blocks)
    print("built; instructions:", n)
